# revision 14
# baseline (speedup 1.0000x reference)
"""GNN message-passing kernel for Trainium2 (8 NeuronCores).

Math: out[e] = agg[src[e]] - x[src[e]] + extra[src[e]] where
agg = segment_sum(x[src], src).  Since segment_sum of x[src] over src is
x * deg (deg = histogram of src), out[e] = y[src[e]] with
y = x * (deg - 1) + extra.

Strategy (edge-parallel, node-range sharded):
- Shard nodes into 8 contiguous ranges of 12500; each edge goes to the core
  owning its src node (edges sorted by (core, src) on host).
- Device computes deg via windowed equality-count histogram, builds y, then
  expands y into the sorted edge stream (out_sorted = repeat(y, deg)) with
  run-indicator matrices and block-diagonal matmuls -- fully structured data
  movement, no indirect DMA.
- Host only shards/sorts/pads inputs and unpads/unsorts the output.
"""

import numpy as np

NC = 8          # cores
NPC = 12500     # nodes per core
T = 98          # node blocks of 128 per core (98*128 = 12544 >= 12500)
TR = T * 128    # padded table rows per core
P = 128
F = 16
NS = 13         # histogram t-slices (t>>3 for t in [0,98))

_COMPILED = {}


def _build(SW, SWIN, reps=0):
    import contextlib
    from concourse import bacc, mybir
    import concourse.tile as tile

    dt = mybir.dt
    nc = bacc.Bacc("TRN2", target_bir_lowering=False, debug=False, num_devices=NC)

    x_d = nc.dram_tensor("xdev", [P, T * F], dt.float32, kind="ExternalInput")
    e_d = nc.dram_tensor("edev", [P, T * F], dt.float32, kind="ExternalInput")
    h_d = nc.dram_tensor("hist", [P, NS * SW], dt.int16, kind="ExternalInput")
    sc_d = nc.dram_tensor("scalcols", [P, T], dt.float32, kind="ExternalInput")
    io_d = nc.dram_tensor("iotarow", [P, SWIN], dt.float32, kind="ExternalInput")
    bm_d = nc.dram_tensor("blockmask", [P, P], dt.float32, kind="ExternalInput")
    lt_d = nc.dram_tensor("ltmat", [P, P], dt.float32, kind="ExternalInput")
    out_d = nc.dram_tensor("out", [T, P, SWIN], dt.float32, kind="ExternalOutput")

    OP = mybir.AluOpType

    with tile.TileContext(nc) as tc:
        with (
            tc.tile_pool(name="pers", bufs=1) as pers,
            tc.tile_pool(name="loop", bufs=4) as loop,
            tc.tile_pool(name="psum", bufs=3, space="PSUM") as psum,
            tc.tile_pool(name="psum2", bufs=2, space="PSUM") as psum2,
            tc.For_i(0, reps, 1) if reps else contextlib.nullcontext(),
        ):
            # ---- load persistent data ----
            x_t = pers.tile([P, T, F], dt.float32)
            e_t = pers.tile([P, T, F], dt.float32)
            h_raw = pers.tile([P, NS * SW], dt.int16)
            scal = pers.tile([P, T], dt.float32)
            iorow = pers.tile([P, SWIN], dt.float32)
            bmask = pers.tile([P, P], dt.float32)
            ltm = pers.tile([P, P], dt.float32)
            nc.sync.dma_start(out=x_t[:, :, :].rearrange("p t f -> p (t f)"),
                              in_=x_d.ap())
            nc.sync.dma_start(out=e_t[:, :, :].rearrange("p t f -> p (t f)"),
                              in_=e_d.ap())
            nc.sync.dma_start(out=h_raw[:], in_=h_d.ap())
            nc.sync.dma_start(out=scal[:], in_=sc_d.ap())
            nc.sync.dma_start(out=iorow[:], in_=io_d.ap())
            nc.sync.dma_start(out=bmask[:], in_=bm_d.ap())
            nc.sync.dma_start(out=ltm[:], in_=lt_d.ap())

            zbias = pers.tile([P, 1], dt.float32)
            nc.gpsimd.memset(zbias[:], 0.0)

            # ---- histogram: deg[p, t] = count(m == 128t + p) ----
            m_f32 = pers.tile([P, NS * SW], dt.float32)
            nc.vector.tensor_copy(m_f32[:], h_raw[:])
            deg = pers.tile([P, T], dt.float32)
            scr = pers.tile([P, SW], dt.float32)
            for t in range(T):
                s = t >> 3
                nc.vector.tensor_scalar(
                    out=scr[:],
                    in0=m_f32[:, s * SW:(s + 1) * SW],
                    scalar1=scal[:, t:t + 1],
                    scalar2=None,
                    op0=OP.is_equal,
                    op1=OP.add,
                    accum_out=deg[:, t:t + 1],
                )

            # ---- y = x * (deg - 1) + extra ----
            degm1 = pers.tile([P, T], dt.float32)
            nc.vector.tensor_scalar_add(degm1[:], deg[:], -1.0)
            y_t = pers.tile([P, T, F], dt.float32)
            nc.vector.tensor_mul(
                y_t[:, :, :], x_t[:, :, :],
                degm1[:, :, None].to_broadcast([P, T, F]))
            nc.vector.tensor_add(y_t[:, :, :], y_t[:, :, :], e_t[:, :, :])

            # ---- cum[p, t] = sum_{k<p within p's 16-block} deg[k, t] ----
            cum_ps = psum2.tile([P, T], dt.float32)
            nc.tensor.matmul(out=cum_ps[:], lhsT=ltm[:], rhs=deg[:],
                             start=True, stop=True)
            cum = pers.tile([P, T], dt.float32)
            nc.vector.tensor_copy(cum[:], cum_ps[:])
            cumend = pers.tile([P, T], dt.float32)
            nc.vector.tensor_add(cumend[:], cum[:], deg[:])

            # ---- main loop over node blocks ----
            for t in range(T):
                # run-end indicator then fused run-start*end
                a2r = loop.tile([P, SWIN], dt.float32, tag="a2r")
                nc.gpsimd.tensor_scalar(
                    out=a2r[:], in0=iorow[:],
                    scalar1=cumend[:, t:t + 1], scalar2=None,
                    op0=OP.is_lt)
                selt = loop.tile([P, SWIN], dt.float32, tag="selt")
                nc.vector.scalar_tensor_tensor(
                    out=selt[:], in0=iorow[:],
                    scalar=cum[:, t:t + 1], in1=a2r[:],
                    op0=OP.is_ge, op1=OP.mult)
                # block-diagonal y slice
                bd = loop.tile([P, P], dt.float32, tag="bd")
                nc.gpsimd.tensor_mul(
                    bd[:].rearrange("p (u f) -> p u f", u=8),
                    y_t[:, t:t + 1, :].to_broadcast([P, 8, F]),
                    bmask[:].rearrange("p (u f) -> p u f", u=8))
                # expand: out[u*16+f, i] = sum_p bd[p, u*16+f] * selt[p, i]
                ops = psum.tile([P, SWIN], dt.float32, tag="ops")
                for lo in range(0, SWIN, 512):
                    hi = min(lo + 512, SWIN)
                    nc.tensor.matmul(out=ops[:, lo:hi], lhsT=bd[:],
                                     rhs=selt[:, lo:hi], start=True, stop=True)
                stage = loop.tile([P, SWIN], dt.float32, tag="stage")
                nc.scalar.activation(
                    stage[:], ops[:],
                    mybir.ActivationFunctionType.Identity, bias=zbias[:])
                nc.sync.dma_start(out=out_d.ap()[t], in_=stage[:])

    nc.compile()
    return nc


def _get_nc(SW, SWIN, reps=0):
    key = (SW, SWIN, reps)
    if key not in _COMPILED:
        _COMPILED[key] = _build(SW, SWIN, reps)
    return _COMPILED[key]


LAST_EXEC_NS = None
LAST_PROFILE = None
LAST_RUN_S = None
LAST_PREP_S = None
LAST_RES = None
LAST_INMAPS = None
LAST_NC = None
LAST_SW = None
LAST_SWIN = None


def bench_device_ns(r1=2000, r2=12000, tries=4):
    """Estimate per-iteration device time by repeating the kernel body
    in-NEFF and differencing wall-clock between two repeat counts."""
    import time as _time
    from concourse.bass_utils import run_bass_kernel_spmd

    assert LAST_INMAPS is not None
    out = {}
    for r in (r1, r2):
        nc_b = _get_nc(LAST_SW, LAST_SWIN, reps=r)
        ts = []
        for _ in range(tries):
            t0 = _time.time()
            run_bass_kernel_spmd(nc_b, LAST_INMAPS, list(range(NC)))
            ts.append(_time.time() - t0)
        out[r] = min(ts)
        print(f"reps={r}: wall times {[f'{t:.2f}' for t in ts]}")
    dev_ns = (out[r2] - out[r1]) / (r2 - r1) * 1e9
    return dev_ns


def kernel(x, extra, edge_index, _trace=False):
    global LAST_EXEC_NS, LAST_PROFILE, LAST_RUN_S, LAST_PREP_S
    global LAST_RES, LAST_INMAPS, LAST_NC
    import time as _time
    from concourse.bass_utils import run_bass_kernel_spmd

    _t0 = _time.time()
    x = np.asarray(x, dtype=np.float32)
    extra = np.asarray(extra, dtype=np.float32)
    src = np.asarray(edge_index)[0].astype(np.int64)
    E = src.shape[0]

    c = src // NPC                       # owning core
    m = src - c * NPC                    # local node id [0, 12500)
    p_h = m & 127
    t_h = m >> 7                         # [0, 98)
    u_h = (m >> 4) & 7                   # 16-row sub-block (stream)

    # ---- histogram input: group edges by (c, p, t-slice), pad with -1 ----
    s_h = t_h >> 3                       # [0, 13)
    gidx = (c * P + p_h) * NS + s_h
    order_h = np.argsort(gidx, kind="stable")
    gcnt = np.bincount(gidx, minlength=NC * P * NS)
    SW = int(np.ceil((gcnt.max() + 1) / 32) * 32)
    gstart = np.zeros(NC * P * NS + 1, np.int64)
    np.cumsum(gcnt, out=gstart[1:])
    pos_h = np.arange(E, dtype=np.int64) - gstart[gidx[order_h]]
    hist = np.full((NC * P * NS, SW), -1, np.int16)
    hist[gidx[order_h], pos_h] = m[order_h].astype(np.int16)
    hist = hist.reshape(NC, P, NS * SW)

    # ---- slot stream layout: block (c, t, u), sorted by m within block ----
    bidx = (c * T + t_h) * 8 + u_h
    key = (bidx << 4) | (m & 15)
    border = np.argsort(key, kind="stable")
    bcnt = np.bincount(bidx, minlength=NC * T * 8)
    SWIN = int(np.ceil((bcnt.max() + 8) / 64) * 64)
    bstart = np.zeros(NC * T * 8 + 1, np.int64)
    np.cumsum(bcnt, out=bstart[1:])
    pos_b = np.arange(E, dtype=np.int64) - bstart[bidx[border]]

    # ---- per-core x/extra tables, interleaved [p, t, f] ----
    def table(a, ci):
        tab = np.zeros((TR, F), np.float32)
        tab[:NPC] = a[ci * NPC:(ci + 1) * NPC]
        return np.ascontiguousarray(
            tab.reshape(T, P, F).transpose(1, 0, 2)).reshape(P, T * F)

    # ---- constants ----
    scalcols = (128.0 * np.arange(T, dtype=np.float32)[None, :]
                + np.arange(P, dtype=np.float32)[:, None]).astype(np.float32)
    iotarow = np.broadcast_to(
        np.arange(SWIN, dtype=np.float32)[None, :], (P, SWIN)).copy()
    pp = np.arange(P)
    blockmask = ((pp[:, None] // 16) == (pp[None, :] // 16)).astype(np.float32)
    ltmat = (((pp[:, None] // 16) == (pp[None, :] // 16))
             & ((pp[:, None] % 16) < (pp[None, :] % 16))).astype(np.float32)

    in_maps = []
    for ci in range(NC):
        in_maps.append({
            "xdev": table(x, ci),
            "edev": table(extra, ci),
            "hist": hist[ci],
            "scalcols": scalcols,
            "iotarow": iotarow,
            "blockmask": blockmask,
            "ltmat": ltmat,
        })
    global LAST_SW, LAST_SWIN
    LAST_SW, LAST_SWIN = SW, SWIN
    nc_b = _get_nc(SW, SWIN)
    LAST_PREP_S = _time.time() - _t0
    _t1 = _time.time()
    res = run_bass_kernel_spmd(nc_b, in_maps, list(range(NC)))
    LAST_RUN_S = _time.time() - _t1
    LAST_RES = res
    LAST_INMAPS = in_maps
    LAST_NC = nc_b

    # ---- unpack: edge at (c, t, u, i) -> out[c][t, u*16:(u+1)*16, i] ----
    arr = np.stack([res.results[ci]["out"] for ci in range(NC)])  # [NC,T,P,SWIN]
    # -> [NC, T, SWIN, 8, 16] so rows are (slot, stream, feature)
    arrT = np.ascontiguousarray(arr.transpose(0, 1, 3, 2)).reshape(
        NC, T, SWIN, 8, F)
    cs = c[border]
    ts = t_h[border]
    us = u_h[border]
    out_sorted = arrT[cs, ts, pos_b, us]           # [E, F]
    result = np.empty((E, F), np.float32)
    result[border] = out_sorted
    return result


# revision 16
# speedup vs baseline: 3.7093x; 3.7093x over previous
"""GNN message-passing kernel for Trainium2 (8 NeuronCores).

Math: out[e] = agg[src[e]] - x[src[e]] + extra[src[e]] where
agg = segment_sum(x[src], src).  Since segment_sum of x[src] over src is
x * deg (deg = histogram of src), out[e] = y[src[e]] with
y = x * (deg - 1) + extra.

Strategy (edge-parallel, node-range sharded):
- Shard nodes into 8 contiguous ranges of 12500; each edge goes to the core
  owning its src node (edges sorted by (core, src) on host).
- Device computes deg via windowed equality-count histogram, builds y, then
  expands y into the sorted edge stream (out_sorted = repeat(y, deg)) with
  run-indicator matrices and block-diagonal matmuls -- fully structured data
  movement, no indirect DMA.
- Host only shards/sorts/pads inputs and unpads/unsorts the output.
"""

import numpy as np

NC = 8          # cores
NPC = 12500     # nodes per core
T = 98          # node blocks of 128 per core (98*128 = 12544 >= 12500)
TR = T * 128    # padded table rows per core
P = 128
F = 16
NS = 13         # histogram t-slices (t>>3 for t in [0,98))

_COMPILED = {}


def _build(SW, SWIN, reps=0):
    import contextlib
    from concourse import bacc, mybir
    import concourse.tile as tile

    dt = mybir.dt
    nc = bacc.Bacc("TRN2", target_bir_lowering=False, debug=False, num_devices=NC)

    x_d = nc.dram_tensor("xdev", [P, T * F], dt.float32, kind="ExternalInput")
    e_d = nc.dram_tensor("edev", [P, T * F], dt.float32, kind="ExternalInput")
    h_d = nc.dram_tensor("hist", [P, NS * SW], dt.int16, kind="ExternalInput")
    sc_d = nc.dram_tensor("scalcols", [P, T], dt.float32, kind="ExternalInput")
    io_d = nc.dram_tensor("iotarow", [P, SWIN], dt.float32, kind="ExternalInput")
    bm_d = nc.dram_tensor("blockmask", [P, P], dt.float32, kind="ExternalInput")
    lt_d = nc.dram_tensor("ltmat", [P, P], dt.float32, kind="ExternalInput")
    out_d = nc.dram_tensor("out", [T, P, SWIN], dt.float32, kind="ExternalOutput")

    OP = mybir.AluOpType

    with tile.TileContext(nc) as tc:
        with (
            tc.tile_pool(name="pers", bufs=1) as pers,
            tc.tile_pool(name="loop", bufs=4) as loop,
            tc.tile_pool(name="psum", bufs=3, space="PSUM") as psum,
            tc.tile_pool(name="psum2", bufs=2, space="PSUM") as psum2,
            tc.For_i(0, reps, 1) if reps else contextlib.nullcontext(),
        ):
            # ---- load persistent data ----
            x_t = pers.tile([P, T, F], dt.float32)
            e_t = pers.tile([P, T, F], dt.float32)
            h_raw = pers.tile([P, NS * SW], dt.int16)
            scal = pers.tile([P, T], dt.float32)
            iorow = pers.tile([P, SWIN], dt.float32)
            bmask = pers.tile([P, P], dt.float32)
            ltm = pers.tile([P, P], dt.float32)
            nc.sync.dma_start(out=x_t[:, :, :].rearrange("p t f -> p (t f)"),
                              in_=x_d.ap())
            nc.sync.dma_start(out=e_t[:, :, :].rearrange("p t f -> p (t f)"),
                              in_=e_d.ap())
            nc.sync.dma_start(out=h_raw[:], in_=h_d.ap())
            nc.sync.dma_start(out=scal[:], in_=sc_d.ap())
            nc.sync.dma_start(out=iorow[:], in_=io_d.ap())
            nc.sync.dma_start(out=bmask[:], in_=bm_d.ap())
            nc.sync.dma_start(out=ltm[:], in_=lt_d.ap())

            zbias = pers.tile([P, 1], dt.float32)
            nc.gpsimd.memset(zbias[:], 0.0)

            # ---- histogram: deg[p, t] = count(m == 128t + p) ----
            m_f32 = pers.tile([P, NS * SW], dt.float32)
            nc.vector.tensor_copy(m_f32[:], h_raw[:])
            deg = pers.tile([P, T], dt.float32)
            scr = pers.tile([P, SW], dt.float32)
            for t in range(T):
                s = t >> 3
                nc.vector.tensor_scalar(
                    out=scr[:],
                    in0=m_f32[:, s * SW:(s + 1) * SW],
                    scalar1=scal[:, t:t + 1],
                    scalar2=None,
                    op0=OP.is_equal,
                    op1=OP.add,
                    accum_out=deg[:, t:t + 1],
                )

            # ---- y = x * (deg - 1) + extra ----
            degm1 = pers.tile([P, T], dt.float32)
            nc.vector.tensor_scalar_add(degm1[:], deg[:], -1.0)
            y_t = pers.tile([P, T, F], dt.float32)
            nc.vector.tensor_mul(
                y_t[:, :, :], x_t[:, :, :],
                degm1[:, :, None].to_broadcast([P, T, F]))
            nc.vector.tensor_add(y_t[:, :, :], y_t[:, :, :], e_t[:, :, :])

            # ---- cum[p, t] = sum_{k<p within p's 16-block} deg[k, t] ----
            cum_ps = psum2.tile([P, T], dt.float32)
            nc.tensor.matmul(out=cum_ps[:], lhsT=ltm[:], rhs=deg[:],
                             start=True, stop=True)
            cum = pers.tile([P, T], dt.float32)
            nc.vector.tensor_copy(cum[:], cum_ps[:])
            cumend = pers.tile([P, T], dt.float32)
            nc.vector.tensor_add(cumend[:], cum[:], deg[:])

            # ---- main loop over node blocks ----
            for t in range(T):
                # run-end indicator then fused run-start*end
                a2r = loop.tile([P, SWIN], dt.float32, tag="a2r")
                nc.vector.tensor_scalar(
                    out=a2r[:], in0=iorow[:],
                    scalar1=cumend[:, t:t + 1], scalar2=None,
                    op0=OP.is_lt)
                selt = loop.tile([P, SWIN], dt.float32, tag="selt")
                nc.vector.scalar_tensor_tensor(
                    out=selt[:], in0=iorow[:],
                    scalar=cum[:, t:t + 1], in1=a2r[:],
                    op0=OP.is_ge, op1=OP.mult)
                # block-diagonal y slice
                bd = loop.tile([P, P], dt.float32, tag="bd")
                nc.vector.tensor_mul(
                    bd[:].rearrange("p (u f) -> p u f", u=8),
                    y_t[:, t:t + 1, :].to_broadcast([P, 8, F]),
                    bmask[:].rearrange("p (u f) -> p u f", u=8))
                # expand: out[u*16+f, i] = sum_p bd[p, u*16+f] * selt[p, i]
                ops = psum.tile([P, SWIN], dt.float32, tag="ops")
                for lo in range(0, SWIN, 512):
                    hi = min(lo + 512, SWIN)
                    nc.tensor.matmul(out=ops[:, lo:hi], lhsT=bd[:],
                                     rhs=selt[:, lo:hi], start=True, stop=True)
                stage = loop.tile([P, SWIN], dt.float32, tag="stage")
                nc.scalar.activation(
                    stage[:], ops[:],
                    mybir.ActivationFunctionType.Identity, bias=zbias[:])
                nc.sync.dma_start(out=out_d.ap()[t], in_=stage[:])

    nc.compile()
    return nc


def _get_nc(SW, SWIN, reps=0):
    key = (SW, SWIN, reps)
    if key not in _COMPILED:
        _COMPILED[key] = _build(SW, SWIN, reps)
    return _COMPILED[key]


LAST_EXEC_NS = None
LAST_PROFILE = None
LAST_RUN_S = None
LAST_PREP_S = None
LAST_RES = None
LAST_INMAPS = None
LAST_NC = None
LAST_SW = None
LAST_SWIN = None


def bench_device_ns(r1=2000, r2=12000, tries=4):
    """Estimate per-iteration device time by repeating the kernel body
    in-NEFF and differencing wall-clock between two repeat counts."""
    import time as _time
    from concourse.bass_utils import run_bass_kernel_spmd

    assert LAST_INMAPS is not None
    out = {}
    for r in (r1, r2):
        nc_b = _get_nc(LAST_SW, LAST_SWIN, reps=r)
        ts = []
        for _ in range(tries):
            t0 = _time.time()
            run_bass_kernel_spmd(nc_b, LAST_INMAPS, list(range(NC)))
            ts.append(_time.time() - t0)
        out[r] = min(ts)
        print(f"reps={r}: wall times {[f'{t:.2f}' for t in ts]}")
    dev_ns = (out[r2] - out[r1]) / (r2 - r1) * 1e9
    return dev_ns


def kernel(x, extra, edge_index, _trace=False):
    global LAST_EXEC_NS, LAST_PROFILE, LAST_RUN_S, LAST_PREP_S
    global LAST_RES, LAST_INMAPS, LAST_NC
    import time as _time
    from concourse.bass_utils import run_bass_kernel_spmd

    _t0 = _time.time()
    x = np.asarray(x, dtype=np.float32)
    extra = np.asarray(extra, dtype=np.float32)
    src = np.asarray(edge_index)[0].astype(np.int64)
    E = src.shape[0]

    c = src // NPC                       # owning core
    m = src - c * NPC                    # local node id [0, 12500)
    p_h = m & 127
    t_h = m >> 7                         # [0, 98)
    u_h = (m >> 4) & 7                   # 16-row sub-block (stream)

    # ---- histogram input: group edges by (c, p, t-slice), pad with -1 ----
    s_h = t_h >> 3                       # [0, 13)
    gidx = (c * P + p_h) * NS + s_h
    order_h = np.argsort(gidx, kind="stable")
    gcnt = np.bincount(gidx, minlength=NC * P * NS)
    SW = int(np.ceil((gcnt.max() + 1) / 32) * 32)
    gstart = np.zeros(NC * P * NS + 1, np.int64)
    np.cumsum(gcnt, out=gstart[1:])
    pos_h = np.arange(E, dtype=np.int64) - gstart[gidx[order_h]]
    hist = np.full((NC * P * NS, SW), -1, np.int16)
    hist[gidx[order_h], pos_h] = m[order_h].astype(np.int16)
    hist = hist.reshape(NC, P, NS * SW)

    # ---- slot stream layout: block (c, t, u), sorted by m within block ----
    bidx = (c * T + t_h) * 8 + u_h
    key = (bidx << 4) | (m & 15)
    border = np.argsort(key, kind="stable")
    bcnt = np.bincount(bidx, minlength=NC * T * 8)
    SWIN = int(np.ceil((bcnt.max() + 8) / 64) * 64)
    bstart = np.zeros(NC * T * 8 + 1, np.int64)
    np.cumsum(bcnt, out=bstart[1:])
    pos_b = np.arange(E, dtype=np.int64) - bstart[bidx[border]]

    # ---- per-core x/extra tables, interleaved [p, t, f] ----
    def table(a, ci):
        tab = np.zeros((TR, F), np.float32)
        tab[:NPC] = a[ci * NPC:(ci + 1) * NPC]
        return np.ascontiguousarray(
            tab.reshape(T, P, F).transpose(1, 0, 2)).reshape(P, T * F)

    # ---- constants ----
    scalcols = (128.0 * np.arange(T, dtype=np.float32)[None, :]
                + np.arange(P, dtype=np.float32)[:, None]).astype(np.float32)
    iotarow = np.broadcast_to(
        np.arange(SWIN, dtype=np.float32)[None, :], (P, SWIN)).copy()
    pp = np.arange(P)
    blockmask = ((pp[:, None] // 16) == (pp[None, :] // 16)).astype(np.float32)
    ltmat = (((pp[:, None] // 16) == (pp[None, :] // 16))
             & ((pp[:, None] % 16) < (pp[None, :] % 16))).astype(np.float32)

    in_maps = []
    for ci in range(NC):
        in_maps.append({
            "xdev": table(x, ci),
            "edev": table(extra, ci),
            "hist": hist[ci],
            "scalcols": scalcols,
            "iotarow": iotarow,
            "blockmask": blockmask,
            "ltmat": ltmat,
        })
    global LAST_SW, LAST_SWIN
    LAST_SW, LAST_SWIN = SW, SWIN
    nc_b = _get_nc(SW, SWIN)
    LAST_PREP_S = _time.time() - _t0
    _t1 = _time.time()
    res = run_bass_kernel_spmd(nc_b, in_maps, list(range(NC)))
    LAST_RUN_S = _time.time() - _t1
    LAST_RES = res
    LAST_INMAPS = in_maps
    LAST_NC = nc_b

    # ---- unpack: edge at (c, t, u, i) -> out[c][t, u*16:(u+1)*16, i] ----
    arr = np.stack([res.results[ci]["out"] for ci in range(NC)])  # [NC,T,P,SWIN]
    # -> [NC, T, SWIN, 8, 16] so rows are (slot, stream, feature)
    arrT = np.ascontiguousarray(arr.transpose(0, 1, 3, 2)).reshape(
        NC, T, SWIN, 8, F)
    cs = c[border]
    ts = t_h[border]
    us = u_h[border]
    out_sorted = arrT[cs, ts, pos_b, us]           # [E, F]
    result = np.empty((E, F), np.float32)
    result[border] = out_sorted
    return result
